# revision 5
# baseline (speedup 1.0000x reference)
"""Trainium2 Bass kernel for nn_Attention_50500225466997.

Computation (per batch): qkv = BN(conv1x1(x)); 4-head attention over L=1024
(DK=32, DH=64); out = attn + BN(dwconv3x3(v)); y = BN(conv1x1(out)).

Strategy (v3):
  - Data-parallel over batch: 16 batches -> 8 NeuronCores, 2 per core.
  - The TRN2 PE clock ramps 0.65 -> 1.2 -> 2.4 GHz with sustained use; a
    dense dependency-free matmul stream runs 512-row fp32r matmuls at
    227 ns vs ~410 ns when the stream has micro-stalls. The whole kernel
    is therefore emitted as one dense PE stream: attention phases carry
    software-pipelined score->exp->AV chains (AV lags scores by 2 tiles)
    with depthwise/pointwise/next-batch-qkv matmuls woven into the gaps.
  - Scalar (ACT) engine does EXP only; all PSUM evacuations and bias adds
    run on DVE via tensor_scalar_add (per-partition bias APs).
  - vT is produced directly as matmul(lhsT=X chunk, rhs=Wv^T) (X is
    stationary), skipping the separate PE transposes; v-bias is folded
    into the pointwise bias (softmax output is shifted by exactly bv).
  - K needs no bias at all (constant-over-l shifts cancel in softmax).
  - Z rides free in the AV matmul via ones columns: per head pair the
    stationary tile is [1 | v_odd | v_even | 1]; bank A = [Z_o; O_o],
    bank B = [O_e; Z_e]. 1/Z via cross-partition reciprocal_approx_fast
    (DVE reads partitions 64:128, writes 0:64) - no staging copies/DMAs.
  - Depthwise 3x3 via 9 permuted-diagonal matmuls accumulating in PSUM.
"""

import numpy as np

import concourse.bass as bass
import concourse.mybir as mybir
import concourse.tile as tile
from concourse import bacc
from concourse.bass_utils import run_bass_kernel_spmd

F32 = mybir.dt.float32
F32R = mybir.dt.float32r
AF = mybir.ActivationFunctionType
OP = mybir.AluOpType

B, CH, HH, WW = 16, 256, 32, 32
L = HH * WW                   # 1024
NH, DK, DH = 4, 32, 64
CQKV = CH + DK * NH * 2       # 512
SCALE = DK ** (-0.5)
NCORES = 8
BL = B // NCORES              # batches per core


def build_bass():
    nc = bacc.Bacc("TRN2", target_bir_lowering=False, debug=False)

    x_d = nc.dram_tensor("x", [BL, CH, L], F32R, kind="ExternalInput")
    wqkvT_d = nc.dram_tensor("wqkvT", [128, 2, CQKV], F32R, kind="ExternalInput")
    bq3_d = nc.dram_tensor("bq3", [128, 3], F32, kind="ExternalInput")
    wpwT_d = nc.dram_tensor("wpwT", [128, 2, CH], F32R, kind="ExternalInput")
    bpw_d = nc.dram_tensor("bpw", [128, 2], F32, kind="ExternalInput")
    diag_d = nc.dram_tensor("diag", [128, 18, 128], F32R, kind="ExternalInput")
    out_d = nc.dram_tensor("out", [BL, CH, L], F32, kind="ExternalOutput")

    with tile.TileContext(nc) as tc, nc.allow_low_precision(reason="fp32r"):
        with (
            tc.tile_pool(name="consts", bufs=1) as consts,
            tc.tile_pool(name="xin", bufs=1) as xin,
            tc.tile_pool(name="qkv", bufs=2) as qkvp,
            tc.tile_pool(name="vt", bufs=1) as vtp,
            tc.tile_pool(name="et", bufs=6) as etp,
            tc.tile_pool(name="o2", bufs=2) as o2p,
            tc.tile_pool(name="small", bufs=2) as smallp,
            tc.tile_pool(name="pad", bufs=1) as padp,
            tc.tile_pool(name="psc", bufs=2, space="PSUM") as psc,
            tc.tile_pool(name="pO", bufs=1, space="PSUM") as pOp,
            tc.tile_pool(name="pwork", bufs=1, space="PSUM") as pwork,
        ):
            # ---------------- constants ----------------
            wqkvT = consts.tile([128, 2, CQKV], F32R)
            # K cols first so the first matmul can start ASAP
            nc.sync.dma_start(wqkvT[:, :, 128:256], wqkvT_d.ap()[:, :, 128:256])
            bq3 = consts.tile([128, 3], F32)
            nc.sync.dma_start(bq3, bq3_d.ap())
            nc.sync.dma_start(wqkvT[:, :, 0:128], wqkvT_d.ap()[:, :, 0:128])

            # x prefetch: b0 halves first (column halves feed mt groups)
            Xall = [[xin.tile([128, L], F32R, name=f"x_b{b}c{ct}", tag=f"x{b}{ct}")
                     for ct in range(2)] for b in range(BL)]
            for ct in range(2):
                nc.sync.dma_start(Xall[0][ct][:, 0:512],
                                  x_d.ap()[0, 128 * ct:128 * ct + 128, 0:512])
            nc.sync.dma_start(wqkvT[:, :, 256:512], wqkvT_d.ap()[:, :, 256:512])
            for ct in range(2):
                nc.sync.dma_start(Xall[0][ct][:, 512:1024],
                                  x_d.ap()[0, 128 * ct:128 * ct + 128, 512:1024])
            for ct in range(2):
                nc.sync.dma_start(Xall[1][ct], x_d.ap()[1, 128 * ct:128 * ct + 128, :])

            # late consts (needed only by dw/pw phases)
            wpwT = consts.tile([128, 2, CH], F32R)
            bpw = consts.tile([128, 2], F32)
            diag = consts.tile([128, 18, 128], F32R)
            nc.sync.dma_start(wpwT, wpwT_d.ap())
            nc.sync.dma_start(bpw, bpw_d.ap())
            nc.sync.dma_start(diag, diag_d.ap())

            # per-batch SBUF tiles (explicit handles; pools give rotation)
            Qa = [qkvp.tile([128, L], F32R, name=f"Qa_{b}", tag="Qa") for b in range(2)]
            Ka = [qkvp.tile([128, L], F32R, name=f"Ka_{b}", tag="Ka") for b in range(2)]
            Vv = [[qkvp.tile([128, L], F32R, name=f"V{ct}_{b}", tag=f"V{ct}")
                   for ct in range(2)] for b in range(2)]
            out2 = [[o2p.tile([128, L], F32R, name=f"o2_{b}{hp}", tag=f"o2{hp}")
                     for hp in range(2)] for b in range(2)]
            # Vt layout per head pair: [ones | v_odd | v_even | ones]
            Vt = [[vtp.tile([128, 8, 256], F32R, name=f"vt_{b}{hp}", tag=f"vt{b}{hp}")
                   for hp in range(2)] for b in range(2)]
            pads = [padp.tile([128, 34, 34], F32R, name=f"pad{ct}", tag=f"pad{ct}")
                    for ct in range(2)]

            # ones blocks for batch-0 Vt tiles first (DVE is needed for
            # evacuations at ~4us; batch-1 memsets emitted later)
            for hp in range(2):
                nc.vector.memset(Vt[0][hp][:, :, 0:64].bitcast(F32), 1.0)
                nc.vector.memset(Vt[0][hp][:, :, 192:256].bitcast(F32), 1.0)

            wtag = [0]

            def work_tile(n, shape=(128, 512), dtype=F32):
                # alternate the two single-buf PSUM work tags during the
                # qkv pre-phase; attention-phase ext groups pass a tag
                t = ("w", "dw")[wtag[0] % 2] if n is None else n
                wtag[0] += 1
                return pwork.tile(list(shape), dtype, name=f"wk{wtag[0]}", tag=t)

            def qkv_group(b, ot, mt, tag=None):
                # ot: 0=Q 1=K 2=V0 3=V1
                ms = slice(512 * mt, 512 * mt + 512)
                pq = work_tile(tag)
                for kt in range(2):
                    nc.tensor.matmul(
                        pq, wqkvT[:, kt, 128 * ot:128 * ot + 128],
                        Xall[b][kt][:, ms], start=(kt == 0), stop=(kt == 1))
                if ot == 1:
                    nc.vector.tensor_copy(Ka[b][:, ms], pq)
                elif ot == 0:
                    nc.vector.tensor_scalar_add(Qa[b][:, ms], pq, bq3[:, 0:1])
                else:
                    nc.vector.tensor_scalar_add(
                        Vv[b][ot - 2][:, ms], pq, bq3[:, ot - 1:ot])

            def vt_group(b, lc, tag=None):
                pv = work_tile(tag)
                for kt in range(2):
                    nc.tensor.matmul(
                        pv[:, 0:256], Xall[b][kt][:, 128 * lc:128 * lc + 128],
                        wqkvT[:, kt, 256:512], start=(kt == 0), stop=(kt == 1))
                for hp in range(2):
                    nc.vector.tensor_copy(
                        Vt[b][hp][:, lc, 64:192], pv[:, 128 * hp:128 * hp + 128])

            def pad_copy(b, ct):
                nc.gpsimd.tensor_copy(
                    pads[ct][:, 1:33, 1:33],
                    Vv[b][ct].rearrange("p (a c) -> p a c", a=32))

            dwp_t = {}

            def dw_taps(b, ct, mt, taps):
                # 3-tap slice of the 9-tap depthwise accumulation
                if taps[0] == 0:
                    dwp_t[(ct, mt)] = work_tile("dw")
                dwp = dwp_t[(ct, mt)]
                for tap in taps:
                    dy, dx = tap // 3, tap % 3
                    r0 = 16 * mt + dy
                    nc.tensor.matmul(
                        dwp, diag[:, 9 * ct + tap, :],
                        pads[ct][:, r0:r0 + 16, dx:dx + 32],
                        start=(tap == 0), stop=(tap == 8),
                        skip_group_check=True)

            def dw_stt(b, ct, mt):
                ms = slice(512 * mt, 512 * mt + 512)
                nc.vector.scalar_tensor_tensor(
                    out=out2[b][ct][:, ms], in0=dwp_t[(ct, mt)], scalar=1.0,
                    in1=out2[b][ct][:, ms], op0=OP.mult, op1=OP.add)

            def pw_group(b, mt, ot):
                ms = slice(512 * mt, 512 * mt + 512)
                pp = work_tile("w")
                for kt in range(2):
                    nc.tensor.matmul(
                        pp, wpwT[:, kt, 128 * ot:128 * ot + 128],
                        out2[b][kt][:, ms], start=(kt == 0), stop=(kt == 1))
                osb = smallp.tile([128, 512], F32, name=f"os{b}{mt}{ot}", tag="os")
                nc.vector.tensor_scalar_add(osb, pp, bpw[:, ot:ot + 1])
                nc.sync.dma_start(out_d.ap()[b, 128 * ot:128 * ot + 128, ms], osb)

            def phase(b, hp, mt, exts, post):
                """Attention phase: 8 score-tiles -> exp -> AV accumulate.
                exts: closures emitting small ext matmul groups, consumed
                at lt=3.. points. post: closures run right after the tail."""
                ms = slice(512 * mt, 512 * mt + 512)
                he, ho = 2 * hp, 2 * hp + 1
                pA = pOp.tile([128, 512], F32, name=f"pa{b}{hp}{mt}", tag="pA")
                pB = pOp.tile([128, 512], F32, name=f"pb{b}{hp}{mt}", tag="pB")
                et_t = {}
                ext_i = [0]

                def do_ext():
                    if ext_i[0] < len(exts):
                        exts[ext_i[0]]()
                        ext_i[0] += 1

                def a_pair(lt):
                    nc.tensor.matmul(
                        pA, Vt[b][hp][:, lt, 0:128], et_t[lt][:, 512:1024],
                        start=(lt == 0), stop=(lt == 7), skip_group_check=True)
                    nc.tensor.matmul(
                        pB, Vt[b][hp][:, lt, 128:256], et_t[lt][:, 0:512],
                        start=(lt == 0), stop=(lt == 7), skip_group_check=True)

                for lt in range(8):
                    ls = slice(128 * lt, 128 * lt + 128)
                    sc = psc.tile([128, 1024], F32, name=f"sc{b}{hp}{mt}{lt}",
                                  tag="sc")
                    nc.tensor.matmul(
                        sc[:, 0:512], Ka[b][32 * he:32 * he + 32, ls],
                        Qa[b][32 * he:32 * he + 32, ms], start=True, stop=True,
                        tile_position=(32 * he, 0))
                    nc.tensor.matmul(
                        sc[:, 512:1024], Ka[b][32 * ho:32 * ho + 32, ls],
                        Qa[b][32 * ho:32 * ho + 32, ms], start=True, stop=True,
                        tile_position=(32 * ho, 0))
                    Et = etp.tile([128, 1024], F32R, name=f"e{b}{hp}{mt}{lt}",
                                  tag="e")
                    nc.scalar.activation(Et, sc, AF.Exp)
                    et_t[lt] = Et
                    if lt >= 2:
                        a_pair(lt - 2)
                    if lt >= 3:
                        do_ext()
                a_pair(6)
                do_ext()
                a_pair(7)
                # tail: out2[0:64] = O_e/Z_e, out2[64:128] = O_o/Z_o.
                # reciprocal_approx_fast can't cross partition bases, plain
                # DVE copies can: pack [Z_e; Z_o] first, then one recip.
                ZA = smallp.tile([128, 512], F32, name=f"za{b}{hp}{mt}", tag="za")
                nc.vector.tensor_copy(ZA[0:64, :], pB[64:128, :])
                nc.vector.tensor_copy(ZA[64:128, :], pA[0:64, :])
                Rz = smallp.tile([128, 512], F32, name=f"rz{b}{hp}{mt}", tag="rz")
                nc.vector.reciprocal_approx_fast(out=Rz, in_=ZA)
                nc.vector.scalar_tensor_tensor(
                    out=out2[b][hp][0:64, ms], in0=pB[0:64, :], scalar=1.0,
                    in1=Rz[0:64, :], op0=OP.mult, op1=OP.mult)
                nc.vector.scalar_tensor_tensor(
                    out=out2[b][hp][64:128, ms], in0=pA[64:128, :], scalar=1.0,
                    in1=Rz[64:128, :], op0=OP.mult, op1=OP.mult)
                while ext_i[0] < len(exts):
                    exts[ext_i[0]]()
                    ext_i[0] += 1
                for p in post:
                    p()

            # ---------------- emission schedule ----------------
            def pre(b):
                for mt in range(2):
                    qkv_group(b, 1, mt)          # K first (scores need full Ka)
                for mt in range(2):
                    qkv_group(b, 0, mt)          # Q
                for lc in range(8):
                    vt_group(b, lc)
                for ot in (2, 3):
                    for mt in range(2):
                        qkv_group(b, ot, mt)
                # batch-1 ones memsets + pad zero-init woven after b0 evacs
                if b == 0:
                    for hp in range(2):
                        nc.vector.memset(Vt[1][hp][:, :, 0:64].bitcast(F32), 1.0)
                        nc.vector.memset(Vt[1][hp][:, :, 192:256].bitcast(F32), 1.0)
                    for ct in range(2):
                        nc.vector.memset(pads[ct].bitcast(F32), 0.0)
                for ct in range(2):
                    pad_copy(b, ct)

            pre(0)

            def dw3(b, ct, mt):
                return [lambda t=t: dw_taps(b, ct, mt, (3 * t, 3 * t + 1, 3 * t + 2))
                        for t in range(3)]

            # batch 0 attention, weaving dw(b0) + batch-1 qkv/vT + pw(b0)
            phase(0, 0, 0, dw3(0, 0, 0) + [lambda: qkv_group(1, 1, 0, "w")],
                  [lambda: dw_stt(0, 0, 0)])
            phase(0, 0, 1, dw3(0, 0, 1) + [lambda: qkv_group(1, 1, 1, "w"),
                                           lambda: qkv_group(1, 0, 0, "w")],
                  [lambda: dw_stt(0, 0, 1)])
            phase(0, 1, 0, dw3(0, 1, 0) + [lambda: qkv_group(1, 0, 1, "w"),
                                           lambda: vt_group(1, 0, "w"),
                                           lambda: vt_group(1, 1, "w")],
                  [lambda: dw_stt(0, 1, 0),
                   lambda: pw_group(0, 0, 0), lambda: pw_group(0, 0, 1)])
            phase(0, 1, 1, dw3(0, 1, 1) + [lambda: vt_group(1, 2, "w"),
                                           lambda: vt_group(1, 3, "w"),
                                           lambda: vt_group(1, 4, "w"),
                                           lambda: vt_group(1, 5, "w")],
                  [lambda: dw_stt(0, 1, 1)])
            # inter-batch block: finish b1 qkv, pw(b0, mt1), pads(b1)
            vt_group(1, 6, "w")
            pw_group(0, 1, 0)
            vt_group(1, 7, "dw")
            pw_group(0, 1, 1)
            for ot in (2, 3):
                for mt in range(2):
                    qkv_group(1, ot, mt)
            for ct in range(2):
                pad_copy(1, ct)

            # batch 1 attention
            phase(1, 0, 0, dw3(1, 0, 0), [lambda: dw_stt(1, 0, 0)])
            phase(1, 0, 1, dw3(1, 0, 1), [lambda: dw_stt(1, 0, 1)])
            phase(1, 1, 0, dw3(1, 1, 0),
                  [lambda: dw_stt(1, 1, 0),
                   lambda: pw_group(1, 0, 0), lambda: pw_group(1, 0, 1)])
            phase(1, 1, 1, dw3(1, 1, 1), [lambda: dw_stt(1, 1, 1)])
            pw_group(1, 1, 0)
            pw_group(1, 1, 1)

    nc.compile()
    return nc


def pack_inputs(w_qkv, s_qkv, b_qkv, w_dw, s_dw, b_dw, w_pw, s_pw, b_pw):
    """Host-side weight packing. Returns dict of constant arrays (shared by
    all cores)."""
    f32 = np.float32
    Wq = (w_qkv[:, :, 0, 0] * s_qkv[:, None]).astype(np.float64)  # [512, 256]
    bq = b_qkv.astype(np.float64).copy()

    # row permutation: [Q(h0..h3) | K(h0..h3) | V(h1,h0,h3,h2)]
    perm = []
    for h in range(NH):
        perm += [h * 128 + d for d in range(32)]           # q
    for h in range(NH):
        perm += [h * 128 + 32 + d for d in range(32)]      # k
    for h in (1, 0, 3, 2):
        perm += [h * 128 + 64 + d for d in range(64)]      # v (pair-swapped)
    perm = np.array(perm)
    Wq = Wq[perm]
    bq = bq[perm]
    # fold attention scale into q (weights AND bias)
    Wq[0:128] *= SCALE
    bq[0:128] *= SCALE

    wqkvT = np.ascontiguousarray(
        Wq.T.reshape(2, 128, CQKV).transpose(1, 0, 2)
    ).astype(f32)  # [128, 2, 512]
    # bq3: col0 = Q bias, col1/2 = V0/V1 bias (V-order); K bias dropped
    # (constant-over-l score shifts cancel in softmax)
    bq3 = np.stack([bq[0:128], bq[256:384], bq[384:512]], axis=1).astype(f32)

    # natural (reference) channel order: c = 64h + d
    bv_nat = b_qkv[np.array([h * 128 + 64 + d for h in range(NH)
                             for d in range(64)])].astype(np.float64)

    Wp = (w_pw[:, :, 0, 0] * s_pw[:, None]).astype(np.float64)     # [256, 256]
    # pw bias absorbs: dw bias, and the v-bias the biasless-vT attention
    # path dropped (softmax output shifts by exactly bv per channel)
    bp = b_pw.astype(np.float64) + Wp @ (b_dw.astype(np.float64) + bv_nat)
    wpwT = np.ascontiguousarray(
        Wp.T.reshape(2, 128, CH).transpose(1, 0, 2)
    ).astype(f32)  # [128, 2, 256]
    bpw = np.ascontiguousarray(bp.reshape(2, 128).T).astype(f32)   # [128, 2]

    wd = (w_dw[:, 0] * s_dw[:, None, None]).astype(f32)            # [256, 3, 3]
    # dw input partitions are in V-order ([h1|h0] then [h3|h2]); output
    # must be natural order -> permuted diagonal (swap 64-halves)
    diag = np.zeros((128, 18, 128), f32)
    vord = np.array([h * 64 + d for h in (1, 0, 3, 2) for d in range(64)])
    for ct in range(2):
        for tap in range(9):
            dy, dx = tap // 3, tap % 3
            for p in range(128):
                c_nat = vord[128 * ct + p]         # natural channel index
                diag[p, 9 * ct + tap, (p + 64) % 128] = wd[c_nat, dy, dx]

    return {"wqkvT": wqkvT, "bq3": bq3, "wpwT": wpwT, "bpw": bpw, "diag": diag}


_NC_CACHE = None


def _get_nc():
    global _NC_CACHE
    if _NC_CACHE is None:
        _NC_CACHE = build_bass()
    return _NC_CACHE


def run(inputs, trace=False):
    """Run the bass kernel on 8 cores. inputs = the reference input dict.
    Returns (full_output [16,256,32,32], BassKernelResults)."""
    x = np.ascontiguousarray(np.asarray(inputs["x"], dtype=np.float32)).reshape(
        B, CH, L
    )
    consts = pack_inputs(
        np.asarray(inputs["w_qkv"], np.float32),
        np.asarray(inputs["s_qkv"], np.float32),
        np.asarray(inputs["b_qkv"], np.float32),
        np.asarray(inputs["w_dw"], np.float32),
        np.asarray(inputs["s_dw"], np.float32),
        np.asarray(inputs["b_dw"], np.float32),
        np.asarray(inputs["w_pw"], np.float32),
        np.asarray(inputs["s_pw"], np.float32),
        np.asarray(inputs["b_pw"], np.float32),
    )
    in_maps = []
    for c in range(NCORES):
        m = dict(consts)
        m["x"] = np.ascontiguousarray(x[c * BL:(c + 1) * BL])
        in_maps.append(m)

    nc = _get_nc()
    res = run_bass_kernel_spmd(
        nc, in_maps, core_ids=list(range(NCORES)), trace=trace
    )
    out = np.concatenate([r["out"] for r in res.results], axis=0)
    return out.reshape(B, CH, HH, WW), res


def kernel(**inputs) -> np.ndarray:
    out, _ = run(inputs, trace=False)
    return out


# revision 17
# speedup vs baseline: 1.2898x; 1.2898x over previous
"""Trainium2 Bass kernel for nn_Attention_50500225466997.

Computation (per batch): qkv = BN(conv1x1(x)); 4-head attention over L=1024
(DK=32, DH=64); out = attn + BN(dwconv3x3(v)); y = BN(conv1x1(out)).

Strategy (v3):
  - Data-parallel over batch: 16 batches -> 8 NeuronCores, 2 per core.
  - The TRN2 PE clock ramps 0.65 -> 1.2 -> 2.4 GHz with sustained use; a
    dense dependency-free matmul stream runs 512-row fp32r matmuls at
    227 ns vs ~410 ns when the stream has micro-stalls. The whole kernel
    is therefore emitted as one dense PE stream: attention phases carry
    software-pipelined score->exp->AV chains (AV lags scores by 2 tiles)
    with depthwise/pointwise/next-batch-qkv matmuls woven into the gaps.
  - Scalar (ACT) engine does EXP only; all PSUM evacuations and bias adds
    run on DVE via tensor_scalar_add (per-partition bias APs).
  - vT is produced directly as matmul(lhsT=X chunk, rhs=Wv^T) (X is
    stationary), skipping the separate PE transposes; v-bias is folded
    into the pointwise bias (softmax output is shifted by exactly bv).
  - K needs no bias at all (constant-over-l shifts cancel in softmax).
  - Z rides free in the AV matmul via ones columns: per head pair the
    stationary tile is [1 | v_odd | v_even | 1]; bank A = [Z_o; O_o],
    bank B = [O_e; Z_e]. 1/Z via cross-partition reciprocal_approx_fast
    (DVE reads partitions 64:128, writes 0:64) - no staging copies/DMAs.
  - Depthwise 3x3 via 9 permuted-diagonal matmuls accumulating in PSUM.
"""

import numpy as np

import concourse.bass as bass
import concourse.mybir as mybir
import concourse.tile as tile
from concourse import bacc
from concourse.bass_utils import run_bass_kernel_spmd

F32 = mybir.dt.float32
F32R = mybir.dt.float32r
AF = mybir.ActivationFunctionType
OP = mybir.AluOpType

B, CH, HH, WW = 16, 256, 32, 32
L = HH * WW                   # 1024
NH, DK, DH = 4, 32, 64
CQKV = CH + DK * NH * 2       # 512
SCALE = DK ** (-0.5)
NCORES = 8
BL = B // NCORES              # batches per core


def build_bass():
    nc = bacc.Bacc("TRN2", target_bir_lowering=False, debug=False)

    x_d = nc.dram_tensor("x", [BL, CH, L], F32R, kind="ExternalInput")
    wqkvT_d = nc.dram_tensor("wqkvT", [128, 2, CQKV], F32R, kind="ExternalInput")
    bq3_d = nc.dram_tensor("bq3", [128, 3], F32, kind="ExternalInput")
    wpwT_d = nc.dram_tensor("wpwT", [128, 2, CH], F32R, kind="ExternalInput")
    bpw_d = nc.dram_tensor("bpw", [128, 2], F32, kind="ExternalInput")
    diag_d = nc.dram_tensor("diag", [128, 18, 128], F32R, kind="ExternalInput")
    out_d = nc.dram_tensor("out", [BL, CH, L], F32, kind="ExternalOutput")

    with tile.TileContext(nc) as tc, nc.allow_low_precision(reason="fp32r"):
        with (
            tc.tile_pool(name="consts", bufs=1) as consts,
            tc.tile_pool(name="xin", bufs=1) as xin,
            tc.tile_pool(name="qkv", bufs=2) as qkvp,
            tc.tile_pool(name="vt", bufs=1) as vtp,
            tc.tile_pool(name="et", bufs=6) as etp,
            tc.tile_pool(name="o2", bufs=2) as o2p,
            tc.tile_pool(name="small", bufs=2) as smallp,
            tc.tile_pool(name="pad", bufs=1) as padp,
            tc.tile_pool(name="psc", bufs=2, space="PSUM") as psc,
            tc.tile_pool(name="pO", bufs=1, space="PSUM") as pOp,
            tc.tile_pool(name="pwork", bufs=1, space="PSUM") as pwork,
        ):
            # ---------------- constants ----------------
            # initial DMAs spread across engine DGE queues so the loads
            # the first matmuls need don't serialize on one ring
            wqkvT = consts.tile([128, 2, CQKV], F32R)
            bq3 = consts.tile([128, 3], F32)
            Xall = [[xin.tile([128, L], F32R, name=f"x_b{b}c{ct}", tag=f"x{b}{ct}")
                     for ct in range(2)] for b in range(BL)]
            # K cols + first x halves first so the first matmul starts ASAP
            nc.sync.dma_start(wqkvT[:, :, 128:256], wqkvT_d.ap()[:, :, 128:256])
            nc.scalar.dma_start(Xall[0][0][:, 0:512],
                                x_d.ap()[0, 0:128, 0:512])
            nc.gpsimd.dma_start(Xall[0][1][:, 0:512],
                                x_d.ap()[0, 128:256, 0:512])
            nc.sync.dma_start(bq3, bq3_d.ap())
            nc.sync.dma_start(wqkvT[:, :, 256:512], wqkvT_d.ap()[:, :, 256:512])
            nc.scalar.dma_start(Xall[0][0][:, 512:1024],
                                x_d.ap()[0, 0:128, 512:1024])
            nc.gpsimd.dma_start(Xall[0][1][:, 512:1024],
                                x_d.ap()[0, 128:256, 512:1024])
            nc.sync.dma_start(wqkvT[:, :, 0:128], wqkvT_d.ap()[:, :, 0:128])
            nc.sync.dma_start(Xall[1][0], x_d.ap()[1, 0:128, :])
            nc.scalar.dma_start(Xall[1][1], x_d.ap()[1, 128:256, :])

            # late consts (needed only by dw/pw phases)
            wpwT = consts.tile([128, 2, CH], F32R)
            bpw = consts.tile([128, 2], F32)
            diag = consts.tile([128, 18, 128], F32R)
            nc.gpsimd.dma_start(wpwT, wpwT_d.ap())
            nc.gpsimd.dma_start(bpw, bpw_d.ap())
            nc.scalar.dma_start(diag, diag_d.ap())

            # per-batch SBUF tiles (explicit handles; pools give rotation)
            Qa = [qkvp.tile([128, L], F32R, name=f"Qa_{b}", tag="Qa") for b in range(2)]
            Ka = [qkvp.tile([128, L], F32R, name=f"Ka_{b}", tag="Ka") for b in range(2)]
            Vv = [[qkvp.tile([128, L], F32R, name=f"V{ct}_{b}", tag=f"V{ct}")
                   for ct in range(2)] for b in range(2)]
            out2 = [[o2p.tile([128, L], F32R, name=f"o2_{b}{hp}", tag=f"o2{hp}")
                     for hp in range(2)] for b in range(2)]
            # Vt layout per head pair: [ones | v_odd | v_even | ones]
            Vt = [[vtp.tile([128, 8, 256], F32R, name=f"vt_{b}{hp}", tag=f"vt{b}{hp}")
                   for hp in range(2)] for b in range(2)]
            pads = [padp.tile([128, 34, 34], F32R, name=f"pad{ct}", tag=f"pad{ct}")
                    for ct in range(2)]

            # ones blocks for batch-0 Vt tiles first (DVE is needed for
            # evacuations at ~4us; batch-1 memsets emitted later)
            for hp in range(2):
                nc.vector.memset(Vt[0][hp][:, :, 0:64].bitcast(F32), 1.0)
                nc.vector.memset(Vt[0][hp][:, :, 192:256].bitcast(F32), 1.0)

            wtag = [0]

            def work_tile():
                # alternate the two single-buf PSUM work tags: group N+1's
                # matmuls never wait on group N's DVE evacuation
                t = ("w", "dw")[wtag[0] % 2]
                wtag[0] += 1
                return pwork.tile([128, 512], F32, name=f"wk{wtag[0]}", tag=t)

            def qkv_group(b, ot, mt):
                # ot: 0=Q 1=K 2=V0 3=V1
                ms = slice(512 * mt, 512 * mt + 512)
                pq = work_tile()
                for kt in range(2):
                    nc.tensor.matmul(
                        pq, wqkvT[:, kt, 128 * ot:128 * ot + 128],
                        Xall[b][kt][:, ms], start=(kt == 0), stop=(kt == 1))
                if ot == 1:
                    nc.vector.tensor_copy(Ka[b][:, ms], pq)
                elif ot == 0:
                    nc.vector.tensor_scalar_add(Qa[b][:, ms], pq, bq3[:, 0:1])
                else:
                    nc.vector.tensor_scalar_add(
                        Vv[b][ot - 2][:, ms], pq, bq3[:, ot - 1:ot])

            def vt_group(b, lc):
                pv = work_tile()
                for kt in range(2):
                    nc.tensor.matmul(
                        pv[:, 0:256], Xall[b][kt][:, 128 * lc:128 * lc + 128],
                        wqkvT[:, kt, 256:512], start=(kt == 0), stop=(kt == 1))
                for hp in range(2):
                    nc.vector.tensor_copy(
                        Vt[b][hp][:, lc, 64:192], pv[:, 128 * hp:128 * hp + 128])

            def pad_copy(b, ct):
                nc.gpsimd.tensor_copy(
                    pads[ct][:, 1:33, 1:33],
                    Vv[b][ct].rearrange("p (a c) -> p a c", a=32))

            dwp_t = {}

            def dw_group(b, ct, mt):
                # contiguous 9-tap depthwise accumulation (one work bank)
                dwp = dwp_t[(ct, mt)] = work_tile()
                for tap in range(9):
                    dy, dx = tap // 3, tap % 3
                    r0 = 16 * mt + dy
                    nc.tensor.matmul(
                        dwp, diag[:, 9 * ct + tap, :],
                        pads[ct][:, r0:r0 + 16, dx:dx + 32],
                        start=(tap == 0), stop=(tap == 8))

            def dw_stt(b, ct, mt):
                ms = slice(512 * mt, 512 * mt + 512)
                nc.vector.scalar_tensor_tensor(
                    out=out2[b][ct][:, ms], in0=dwp_t[(ct, mt)], scalar=1.0,
                    in1=out2[b][ct][:, ms], op0=OP.mult, op1=OP.add)

            def pw_group(b, mt, ot):
                ms = slice(512 * mt, 512 * mt + 512)
                pp = work_tile()
                for kt in range(2):
                    nc.tensor.matmul(
                        pp, wpwT[:, kt, 128 * ot:128 * ot + 128],
                        out2[b][kt][:, ms], start=(kt == 0), stop=(kt == 1))
                osb = smallp.tile([128, 512], F32, name=f"os{b}{mt}{ot}", tag="os")
                nc.vector.tensor_scalar_add(osb, pp, bpw[:, ot:ot + 1])
                nc.sync.dma_start(out_d.ap()[b, 128 * ot:128 * ot + 128, ms], osb)

            def phase(b, hp, mt, exts, post):
                """Attention phase: 8 score-tiles -> exp -> AV accumulate
                (AV lags scores by 2 tiles to cover EXP latency). exts:
                closures emitting ext matmul groups, one consumed per
                score tile. post: closures run right after the tail."""
                ms = slice(512 * mt, 512 * mt + 512)
                he, ho = 2 * hp, 2 * hp + 1
                pA = pOp.tile([128, 512], F32, name=f"pa{b}{hp}{mt}", tag="pA")
                pB = pOp.tile([128, 512], F32, name=f"pb{b}{hp}{mt}", tag="pB")
                et_t = {}
                ext_i = [0]

                def do_ext():
                    if ext_i[0] < len(exts):
                        exts[ext_i[0]]()
                        ext_i[0] += 1

                def a_pair(lt):
                    nc.tensor.matmul(
                        pA, Vt[b][hp][:, lt, 0:128], et_t[lt][:, 512:1024],
                        start=(lt == 0), stop=(lt == 7), skip_group_check=True)
                    nc.tensor.matmul(
                        pB, Vt[b][hp][:, lt, 128:256], et_t[lt][:, 0:512],
                        start=(lt == 0), stop=(lt == 7), skip_group_check=True)

                for lt in range(8):
                    ls = slice(128 * lt, 128 * lt + 128)
                    sc = psc.tile([128, 1024], F32, name=f"sc{b}{hp}{mt}{lt}",
                                  tag="sc")
                    nc.tensor.matmul(
                        sc[:, 0:512], Ka[b][32 * he:32 * he + 32, ls],
                        Qa[b][32 * he:32 * he + 32, ms], start=True, stop=True,
                        tile_position=(32 * he, 0))
                    nc.tensor.matmul(
                        sc[:, 512:1024], Ka[b][32 * ho:32 * ho + 32, ls],
                        Qa[b][32 * ho:32 * ho + 32, ms], start=True, stop=True,
                        tile_position=(32 * ho, 0))
                    Et = etp.tile([128, 1024], F32R, name=f"e{b}{hp}{mt}{lt}",
                                  tag="e")
                    nc.scalar.activation(Et, sc, AF.Exp)
                    et_t[lt] = Et
                    if lt >= 8 - len(exts):
                        do_ext()
                    if lt >= 2:
                        a_pair(lt - 2)
                a_pair(6)
                a_pair(7)
                # tail: out2[0:64] = O_e/Z_e, out2[64:128] = O_o/Z_o.
                # reciprocal_approx_fast can't cross partition bases, plain
                # DVE copies can: pack [Z_e; Z_o] first, then one recip.
                ZA = smallp.tile([128, 512], F32, name=f"za{b}{hp}{mt}", tag="za")
                nc.vector.tensor_copy(ZA[0:64, :], pB[64:128, :])
                nc.vector.tensor_copy(ZA[64:128, :], pA[0:64, :])
                Rz = smallp.tile([128, 512], F32, name=f"rz{b}{hp}{mt}", tag="rz")
                nc.vector.reciprocal_approx_fast(out=Rz, in_=ZA)
                nc.vector.scalar_tensor_tensor(
                    out=out2[b][hp][0:64, ms], in0=pB[0:64, :], scalar=1.0,
                    in1=Rz[0:64, :], op0=OP.mult, op1=OP.mult)
                nc.vector.scalar_tensor_tensor(
                    out=out2[b][hp][64:128, ms], in0=pA[64:128, :], scalar=1.0,
                    in1=Rz[64:128, :], op0=OP.mult, op1=OP.mult)
                while ext_i[0] < len(exts):
                    exts[ext_i[0]]()
                    ext_i[0] += 1
                for p in post:
                    p()

            # ---------------- emission schedule ----------------
            # minimal pre-head: just enough for phase(0,0,0) to start;
            # the rest of b0's qkv and all of b1's prep ride as ext slots
            for mt in range(2):
                qkv_group(0, 1, mt)              # K (scores need full Ka)
            qkv_group(0, 0, 0)                   # Q mt0
            for lc in range(4):
                vt_group(0, lc)
            # batch-1 ones memsets + pad zero-init behind the b0 evacs
            for hp in range(2):
                nc.vector.memset(Vt[1][hp][:, :, 0:64].bitcast(F32), 1.0)
                nc.vector.memset(Vt[1][hp][:, :, 192:256].bitcast(F32), 1.0)
            for ct in range(2):
                nc.vector.memset(pads[ct].bitcast(F32), 0.0)

            def pads_ready(b):
                for ct in range(2):
                    pad_copy(b, ct)

            G = lambda f, *a: (lambda: f(*a))
            phase(0, 0, 0,
                  [G(qkv_group, 0, 2, 0), G(qkv_group, 0, 0, 1),
                   G(qkv_group, 0, 2, 1), G(vt_group, 0, 4),
                   G(vt_group, 0, 5), G(vt_group, 0, 6), G(vt_group, 0, 7),
                   G(qkv_group, 0, 3, 0), G(qkv_group, 0, 3, 1),
                   G(pads_ready, 0)],
                  [])
            phase(0, 0, 1,
                  [G(qkv_group, 1, 1, 0), G(qkv_group, 1, 1, 1),
                   G(qkv_group, 1, 0, 0), G(qkv_group, 1, 0, 1),
                   G(vt_group, 1, 0), G(vt_group, 1, 1), G(vt_group, 1, 2),
                   G(dw_group, 0, 0, 0)],
                  [lambda: dw_stt(0, 0, 0)])
            phase(0, 1, 0,
                  [G(vt_group, 1, 3), G(vt_group, 1, 4), G(vt_group, 1, 5),
                   G(vt_group, 1, 6), G(vt_group, 1, 7),
                   G(dw_group, 0, 0, 1), G(dw_group, 0, 1, 0),
                   G(qkv_group, 1, 2, 0)],
                  [lambda: dw_stt(0, 0, 1), lambda: dw_stt(0, 1, 0),
                   lambda: pw_group(0, 0, 0), lambda: pw_group(0, 0, 1)])
            phase(0, 1, 1,
                  [G(qkv_group, 1, 2, 1), G(qkv_group, 1, 3, 0),
                   G(qkv_group, 1, 3, 1), G(dw_group, 0, 1, 1)],
                  [lambda: dw_stt(0, 1, 1), lambda: pads_ready(1),
                   lambda: pw_group(0, 1, 0), lambda: pw_group(0, 1, 1)])

            # batch 1 attention (fully prepped during b0's phases)
            phase(1, 0, 0, [G(dw_group, 1, 0, 0)], [lambda: dw_stt(1, 0, 0)])
            phase(1, 0, 1, [G(dw_group, 1, 0, 1)], [lambda: dw_stt(1, 0, 1)])
            phase(1, 1, 0, [G(dw_group, 1, 1, 0)],
                  [lambda: dw_stt(1, 1, 0),
                   lambda: pw_group(1, 0, 0), lambda: pw_group(1, 0, 1)])
            phase(1, 1, 1, [G(dw_group, 1, 1, 1)],
                  [lambda: dw_stt(1, 1, 1),
                   lambda: pw_group(1, 1, 0), lambda: pw_group(1, 1, 1)])

    nc.compile()
    return nc


def pack_inputs(w_qkv, s_qkv, b_qkv, w_dw, s_dw, b_dw, w_pw, s_pw, b_pw):
    """Host-side weight packing. Returns dict of constant arrays (shared by
    all cores)."""
    f32 = np.float32
    Wq = (w_qkv[:, :, 0, 0] * s_qkv[:, None]).astype(np.float64)  # [512, 256]
    bq = b_qkv.astype(np.float64).copy()

    # row permutation: [Q(h0..h3) | K(h0..h3) | V(h1,h0,h3,h2)]
    perm = []
    for h in range(NH):
        perm += [h * 128 + d for d in range(32)]           # q
    for h in range(NH):
        perm += [h * 128 + 32 + d for d in range(32)]      # k
    for h in (1, 0, 3, 2):
        perm += [h * 128 + 64 + d for d in range(64)]      # v (pair-swapped)
    perm = np.array(perm)
    Wq = Wq[perm]
    bq = bq[perm]
    # fold attention scale into q (weights AND bias)
    Wq[0:128] *= SCALE
    bq[0:128] *= SCALE

    wqkvT = np.ascontiguousarray(
        Wq.T.reshape(2, 128, CQKV).transpose(1, 0, 2)
    ).astype(f32)  # [128, 2, 512]
    # bq3: col0 = Q bias, col1/2 = V0/V1 bias (V-order); K bias dropped
    # (constant-over-l score shifts cancel in softmax)
    bq3 = np.stack([bq[0:128], bq[256:384], bq[384:512]], axis=1).astype(f32)

    # natural (reference) channel order: c = 64h + d
    bv_nat = b_qkv[np.array([h * 128 + 64 + d for h in range(NH)
                             for d in range(64)])].astype(np.float64)

    Wp = (w_pw[:, :, 0, 0] * s_pw[:, None]).astype(np.float64)     # [256, 256]
    # pw bias absorbs: dw bias, and the v-bias the biasless-vT attention
    # path dropped (softmax output shifts by exactly bv per channel)
    bp = b_pw.astype(np.float64) + Wp @ (b_dw.astype(np.float64) + bv_nat)
    wpwT = np.ascontiguousarray(
        Wp.T.reshape(2, 128, CH).transpose(1, 0, 2)
    ).astype(f32)  # [128, 2, 256]
    bpw = np.ascontiguousarray(bp.reshape(2, 128).T).astype(f32)   # [128, 2]

    wd = (w_dw[:, 0] * s_dw[:, None, None]).astype(f32)            # [256, 3, 3]
    # dw input partitions are in V-order ([h1|h0] then [h3|h2]); output
    # must be natural order -> permuted diagonal (swap 64-halves)
    diag = np.zeros((128, 18, 128), f32)
    vord = np.array([h * 64 + d for h in (1, 0, 3, 2) for d in range(64)])
    for ct in range(2):
        for tap in range(9):
            dy, dx = tap // 3, tap % 3
            for p in range(128):
                c_nat = vord[128 * ct + p]         # natural channel index
                diag[p, 9 * ct + tap, (p + 64) % 128] = wd[c_nat, dy, dx]

    return {"wqkvT": wqkvT, "bq3": bq3, "wpwT": wpwT, "bpw": bpw, "diag": diag}


_NC_CACHE = None


def _get_nc():
    global _NC_CACHE
    if _NC_CACHE is None:
        _NC_CACHE = build_bass()
    return _NC_CACHE


def run(inputs, trace=False):
    """Run the bass kernel on 8 cores. inputs = the reference input dict.
    Returns (full_output [16,256,32,32], BassKernelResults)."""
    x = np.ascontiguousarray(np.asarray(inputs["x"], dtype=np.float32)).reshape(
        B, CH, L
    )
    consts = pack_inputs(
        np.asarray(inputs["w_qkv"], np.float32),
        np.asarray(inputs["s_qkv"], np.float32),
        np.asarray(inputs["b_qkv"], np.float32),
        np.asarray(inputs["w_dw"], np.float32),
        np.asarray(inputs["s_dw"], np.float32),
        np.asarray(inputs["b_dw"], np.float32),
        np.asarray(inputs["w_pw"], np.float32),
        np.asarray(inputs["s_pw"], np.float32),
        np.asarray(inputs["b_pw"], np.float32),
    )
    in_maps = []
    for c in range(NCORES):
        m = dict(consts)
        m["x"] = np.ascontiguousarray(x[c * BL:(c + 1) * BL])
        in_maps.append(m)

    nc = _get_nc()
    res = run_bass_kernel_spmd(
        nc, in_maps, core_ids=list(range(NCORES)), trace=trace
    )
    out = np.concatenate([r["out"] for r in res.results], axis=0)
    return out.reshape(B, CH, HH, WW), res


def kernel(**inputs) -> np.ndarray:
    out, _ = run(inputs, trace=False)
    return out
